# revision 17
# baseline (speedup 1.0000x reference)
"""GCN (3-layer) + mean-pool + MLP head on 8 Trainium2 NeuronCores.

Strategy (data-parallel over dst nodes):
- Nodes are partitioned into 8 contiguous ranges (one per core). Each core owns
  all edges whose dst falls in its range (plus self-loops), so the scatter side
  of message passing is core-local.
- Per layer: each core computes h = x @ W for its own nodes (bf16) tile by
  tile DURING the previous layer's finish sweep; each quarter's shard is
  DMA'd + AllGather'd the moment its last tile is ready, so collectives fully
  overlap edge processing.
- Per-edge messages h[src] are fetched with dma_gather (SWDGE indexed DMA,
  256B rows). Gather calls are merged across dst-tile buckets (up to MAXNI
  indices), rotate across 4 SWDGE queues (one descriptor ring each, sized via
  dynamic_dma_scratch_size), and pass per-core true index counts via
  num_idxs_reg (call-tail padding is -1 and skipped by the Q7 loop).
- Segment-sum by dst runs on the tensor engine: for each 128-edge chunk a
  host-built selection matrix Sel[e, slot] = norm[e] * (slot[e] == s) (bf16,
  streamed from DRAM over the scalar-engine HWDGE path) is matmul'd
  (Sel^T @ msgs) into a PSUM tile per 128-node dst tile; per-quarter partials
  are drained into an SBUF accumulator.
- All tensor-engine operands are bf16 (weights, activations, selections);
  accumulation stays f32 in PSUM.
- Graph mean-pool: per-core partial sums via matmul against a host-built
  node->graph selection, AllReduce, then the tiny MLP head runs redundantly
  on every core.
- The SAME NEFF runs on all 8 cores (SPMD): per-(tile, quarter) edge counts
  are padded to the max across cores so the instruction schedule is identical
  everywhere; padded slots have all-zero sel columns so they contribute
  nothing.
- int16 gather indices only reach 32767, so each quarter table keeps its own
  index base (quarter size * 8 cores <= 32768 rows).
"""

import os
import numpy as np
import ml_dtypes

CORES = 8
NGRP = 4      # table quarters
MAXNI = int(os.environ.get("K_MAXNI", "2048"))  # max indices per gather call
NQ = int(os.environ.get("K_NQ", "4"))   # SWDGE queues, round-robin
FULLCNT = os.environ.get("K_FULLCNT") == "1"  # disable num_idxs_reg trick
NOREG = os.environ.get("K_NOREG") == "1"      # pass ni as plain int
SELSYNC = os.environ.get("K_SELSYNC") == "1"  # sel stream on sync engine
MSG_BUFS = 6
bf16_np = ml_dtypes.bfloat16


# ----------------------------------------------------------------- host prep

def _preprocess(pos, edge_index, batch, num_graphs):
    N, D = pos.shape
    G = int(num_graphs)
    assert N % CORES == 0, N
    npc = N // CORES                       # real nodes per core
    TR = (npc + 127) // 128                # real dst tiles per core
    TQ = (TR + NGRP - 1) // NGRP           # tiles per quarter
    T = TQ * NGRP                          # padded tile count
    npc_pad = T * 128
    qsize = TQ * 128                       # rows per core per quarter
    tq = qsize * CORES                     # rows per quarter table
    assert tq <= 32768

    src = np.concatenate([edge_index[0], np.arange(N, dtype=np.int64)])
    dst = np.concatenate([edge_index[1], np.arange(N, dtype=np.int64)])
    deg = np.bincount(dst, minlength=N).astype(np.float32)
    dinv = (1.0 / np.sqrt(np.maximum(deg, 1e-12))).astype(np.float32)
    norm = dinv[src] * dinv[dst]

    core_of = dst // npc
    loc = dst - core_of * npc
    tile_of = loc // 128
    slot_of = loc - tile_of * 128

    # remapped quarter-table row of each node
    c_n = np.arange(N, dtype=np.int64) // npc
    i_n = np.arange(N, dtype=np.int64) - c_n * npc
    q_n = i_n // qsize
    row_n = c_n * qsize + (i_n - q_n * qsize)   # row within quarter table
    grp = q_n[src]
    idx16 = row_n[src]

    # bucket edges by (group, tile, core) -- group-major device sweep order
    order = np.lexsort((idx16, slot_of, core_of, tile_of, grp))
    core_s = core_of[order]
    tile_s = tile_of[order]
    grp_s = grp[order]
    idx_s = idx16[order]
    slot_s = slot_of[order]
    norm_s = norm[order]

    key = (grp_s * TR + tile_s) * CORES + core_s
    nk = NGRP * TR * CORES
    cnt = np.bincount(key, minlength=nk).reshape(NGRP, TR, CORES)
    M = cnt.max(axis=2)                    # [NGRP, TR] padded bucket sizes
    M = ((M + 127) // 128) * 128
    tot = int(M.sum())
    nchunk = tot // 128
    Mc = (M // 128).astype(np.int64)       # chunks per bucket

    # global edge offset of each bucket, group-major
    boff = np.zeros((NGRP, TR), dtype=np.int64)
    off = 0
    for g in range(NGRP):
        for t in range(TR):
            boff[g, t] = off
            off += int(M[g, t])
    assert off == tot

    # merged gather calls: consecutive chunks within a quarter, <= MAXNI
    calls = []                             # (g, off, ni)
    for g in range(NGRP):
        gstart = int(boff[g, 0])
        gend = gstart + int(M[g].sum())
        pos_ = gstart
        while pos_ < gend:
            ni = min(MAXNI, gend - pos_)
            calls.append((g, pos_, ni))
            pos_ += ni
    ncalls = len(calls)
    call_of_chunk = np.zeros(nchunk, dtype=np.int64)
    call_chunk0 = np.zeros(ncalls, dtype=np.int64)
    for k, (g, coff, ni) in enumerate(calls):
        call_of_chunk[coff // 128:(coff + ni) // 128] = k
        call_chunk0[k] = coff // 128

    starts = np.zeros(nk + 1, dtype=np.int64)
    np.cumsum(np.bincount(key, minlength=nk), out=starts[1:])
    idx_in = np.full((CORES, tot), -1, dtype=np.int64)
    slot_in = np.zeros((CORES, tot), dtype=np.int64)
    norm_in = np.zeros((CORES, tot), dtype=np.float32)
    for c in range(CORES):
        for g in range(NGRP):
            for t in range(TR):
                k = (g * TR + t) * CORES + c
                s, e = starts[k], starts[k + 1]
                o = boff[g, t]
                idx_in[c, o:o + (e - s)] = idx_s[s:e]
                slot_in[c, o:o + (e - s)] = slot_s[s:e]
                norm_in[c, o:o + (e - s)] = norm_s[s:e]

    # per-call true counts: valid indices must be a prefix of the call window
    # (ucode stops at the first negative), so interior bucket padding becomes
    # index 0 (gathers row 0, killed by zero sel columns); only the tail of
    # each call keeps -1 and is skipped.
    counts_inp = np.zeros((CORES, 1, ncalls), dtype=np.int32)
    for c in range(CORES):
        for k, (g, coff, ni) in enumerate(calls):
            w = idx_in[c, coff:coff + ni]
            valid = np.nonzero(w >= 0)[0]
            last = int(valid[-1]) + 1 if valid.size else 0
            last = max(last, min(16, ni))
            w[:last][w[:last] < 0] = 0
            counts_inp[c, 0, k] = last
    if FULLCNT:
        idx_in[idx_in < 0] = 0
        for k, (g, coff, ni) in enumerate(calls):
            counts_inp[:, 0, k] = ni

    # wrap indices per call: position i of a call reads tile16[i%16, i//16]
    idx_wrapped = np.zeros((CORES, 16, tot // 16), dtype=np.int16)
    for (g, coff, ni) in calls:
        seg = idx_in[:, coff:coff + ni]
        w = seg.reshape(CORES, ni // 16, 16).transpose(0, 2, 1)
        idx_wrapped[:, :, coff // 16:(coff + ni) // 16] = w.astype(np.int16)
    idxs_inp = np.tile(idx_wrapped, (1, 8, 1))

    # host-built selection matrices: sel[e, slot] = norm[e] one-hot, stored
    # pre-wrapped [128, nchunk*128] bf16 so each partition's stream per call
    # is one contiguous descriptor.
    ee = np.arange(tot, dtype=np.int64)
    bidx = ee // 128
    pidx = ee % 128
    sel = np.zeros((CORES, 128, nchunk, 128), dtype=bf16_np)
    for c in range(CORES):
        sel[c, pidx, bidx, slot_in[c]] = norm_in[c].astype(bf16_np)
    sel_inp = sel.reshape(CORES, 128, nchunk * 128)

    # pos gather table, quarter-remapped order, bf16 256B rows
    pos_pad = np.zeros((NGRP * tq, 128), dtype=bf16_np)
    pos_pad[q_n * tq + row_n, :D] = np.asarray(pos, dtype=np.float32)

    poolsel = np.zeros((CORES, TR, 128, 64 if G <= 64 else G), dtype=bf16_np)
    GP = poolsel.shape[3]
    b = np.asarray(batch, dtype=np.int64)
    for c in range(CORES):
        nodes = np.arange(npc, dtype=np.int64)
        gids = b[c * npc + nodes]
        poolsel[c, nodes // 128, nodes % 128, gids] = 1.0
    cnt_g = np.bincount(b, minlength=GP).astype(np.float32)
    invcnt = (1.0 / np.maximum(cnt_g, 1.0)).astype(np.float32).reshape(GP, 1)

    meta = dict(N=N, D=D, G=G, GP=GP, npc=npc, TR=TR, TQ=TQ, T=T,
                npc_pad=npc_pad, qsize=qsize, tq=tq, tot=tot,
                nchunk=max(nchunk, 1), ncalls=ncalls,
                Mc=Mc.tolist(), boff=boff.tolist(),
                calls=calls, call_of_chunk=call_of_chunk.tolist(),
                call_chunk0=call_chunk0.tolist())
    data = dict(idxs=idxs_inp, sel=sel_inp, counts=counts_inp,
                pos_pad=pos_pad, poolsel=poolsel, invcnt=invcnt)
    return meta, data


# ------------------------------------------------------------- device build

def _build(meta, H, C, reps=1):
    import concourse.bacc as bacc
    import concourse.mybir as mybir
    from concourse.tile import TileContext
    from concourse.library_config import mlp as mlp_lib

    f32 = mybir.dt.float32
    bf16 = mybir.dt.bfloat16
    i16 = mybir.dt.int16
    i32 = mybir.dt.int32
    AF = mybir.ActivationFunctionType

    D = meta["D"]
    GP = meta["GP"]
    TR = meta["TR"]
    TQ = meta["TQ"]
    T = meta["T"]
    qsize = meta["qsize"]
    tq = meta["tq"]
    tot = meta["tot"]
    nchunk = meta["nchunk"]
    ncalls = meta["ncalls"]
    Mc = meta["Mc"]
    boff = meta["boff"]
    calls = meta["calls"]
    call_of_chunk = meta["call_of_chunk"]
    call_chunk0 = meta["call_chunk0"]
    npc_pad = meta["npc_pad"]
    HC = C
    HH = H // 2
    MB = MAXNI // 128

    nc = bacc.Bacc("TRN2", target_bir_lowering=False, debug=False,
                   num_devices=CORES, num_swdge_queues=NQ,
                   dynamic_dma_scratch_size=16 * MAXNI)

    pos_pad_d = nc.dram_tensor("pos_pad", [NGRP * tq, 128], bf16, kind="ExternalInput")
    idxs_d = nc.dram_tensor("idxs", [128, tot // 16], i16, kind="ExternalInput")
    sel_d = nc.dram_tensor("sel", [128, nchunk * 128], bf16, kind="ExternalInput")
    counts_d = nc.dram_tensor("counts", [1, ncalls], i32, kind="ExternalInput")
    poolsel_d = nc.dram_tensor("poolsel", [TR, 128, GP], bf16, kind="ExternalInput")
    invcnt_d = nc.dram_tensor("invcnt", [GP, 1], f32, kind="ExternalInput")
    ident_d = nc.dram_tensor("ident", [128, 128], f32, kind="ExternalInput")
    identb_d = nc.dram_tensor("identb", [128, 128], bf16, kind="ExternalInput")
    ones_d = nc.dram_tensor("ones", [1, 128], bf16, kind="ExternalInput")
    wdecl = (("W1", [D, H]), ("W2", [H, H]), ("W3", [H, H]), ("Wl1", [H, HH]),
             ("Wl2", [HH, HC]), ("b1", [1, H]), ("b2", [1, H]), ("b3", [1, H]),
             ("bl1", [1, HH]), ("bl2", [1, HC]))
    wd = {nm: nc.dram_tensor(nm, shp, bf16, kind="ExternalInput")
          for nm, shp in wdecl}
    out_d = nc.dram_tensor("out", [GP, HC], f32, kind="ExternalOutput")

    hb = {}
    ha = {}
    for l in (2, 3):
        hb[l] = [nc.dram_tensor(f"hbounce{l}{q}", [qsize, H], bf16)
                 for q in range(NGRP)]
        ha[l] = [nc.dram_tensor(f"hall{l}{q}", [tq, H], bf16,
                                addr_space="Shared") for q in range(NGRP)]
    pool_b = nc.dram_tensor("pool_b", [GP, H], f32)
    pool_r = nc.dram_tensor("pool_r", [GP, H], f32, addr_space="Shared")

    # finish-tile t -> quarters whose stage DMA + AllGather fire after it
    qfire = {}
    for q in range(NGRP):
        if q * TQ >= TR:
            qfire.setdefault(TR - 1, []).append(q)
        else:
            qfire.setdefault(min((q + 1) * TQ, TR) - 1, []).append(q)

    with TileContext(nc) as tc:
        nc.gpsimd.load_library(mlp_lib)
        with (
            tc.tile_pool(name="const", bufs=1) as constp,
            tc.tile_pool(name="idx", bufs=1) as idxp,
            tc.tile_pool(name="xt", bufs=2) as xtp,
            tc.tile_pool(name="acc", bufs=1) as accp,
            tc.tile_pool(name="stage", bufs=1) as stagep,
            tc.tile_pool(name="msg", bufs=MSG_BUFS) as msgp,
            tc.tile_pool(name="sel", bufs=4) as selp,
            tc.tile_pool(name="xtile", bufs=3) as xtilep,
            tc.tile_pool(name="small", bufs=4) as smallp,
            tc.tile_pool(name="psum_seg", bufs=3, space="PSUM") as psum_seg,
            tc.tile_pool(name="psum_tr", bufs=1, space="PSUM") as psum_tr,
            tc.tile_pool(name="psum_h", bufs=2, space="PSUM") as psum_h,
        ):
            ident = constp.tile([128, 128], f32)
            nc.sync.dma_start(out=ident[:], in_=ident_d[:, :])
            identb = constp.tile([128, 128], bf16)
            nc.sync.dma_start(out=identb[:], in_=identb_d[:, :])
            ones = constp.tile([1, 128], bf16)
            nc.sync.dma_start(out=ones[:], in_=ones_d[:, :])
            Ws = {}
            for nm, shp in wdecl:
                w = constp.tile(shp, bf16, tag=f"w_{nm}")
                nc.sync.dma_start(out=w[:], in_=wd[nm][:, :])
                Ws[nm] = w
            poolsel = constp.tile([128, TR, GP], bf16)
            nc.sync.dma_start(out=poolsel[:],
                              in_=poolsel_d.ap().rearrange("t p g -> p t g"))
            invcnt = constp.tile([GP, 1], f32)
            nc.sync.dma_start(out=invcnt[:], in_=invcnt_d[:, :])
            idxs = idxp.tile([128, tot // 16], i16)
            nc.sync.dma_start(out=idxs[:], in_=idxs_d[:, :])
            counts = idxp.tile([1, ncalls], i32)
            nc.sync.dma_start(out=counts[:], in_=counts_d[:, :])

            # warm all msg buffers: gathers skip slots past the per-core edge
            # count, and stale-SBUF NaN bit patterns would poison 0*garbage.
            # The warmed tiles are consumed by the first gather calls so the
            # memsets have live uses (DCE drops unread tiles).
            warm = []
            for _ in range(MSG_BUFS):
                mm = msgp.tile([128, MB, 128], bf16, tag="msgb")
                nc.vector.memset(mm[:], 0.0)
                warm.append(mm)

            seleng = nc.sync if SELSYNC else nc.scalar

            def edge_phase(layer, tables, b_name, finish_tile):
                """Group-major sweep; finish_tile(t, acc_slice) after quarter 3."""
                W_ = H if layer > 1 else D
                acc = accp.tile([128, TR, W_], f32,
                                tag="acc" if layer > 1 else "acc1")
                call_tiles = {}

                def emit_call(k):
                    g, coff, ni = calls[k]
                    nbk = ni // 128
                    m = warm.pop(0) if warm else msgp.tile(
                        [128, MB, 128], bf16, tag="msgb")
                    if NOREG:
                        cnt_reg = ni
                    else:
                        cnt_reg = nc.gpsimd.value_load(
                            counts[0:1, k:k + 1])
                    nc.gpsimd.dma_gather(
                        m[:, 0:nbk, :], tables[g][:, :],
                        idxs[:, coff // 16:(coff + ni) // 16],
                        ni, cnt_reg, 128, queue_num=k % NQ)
                    st = selp.tile([128, MB * 128], bf16, tag="selb")
                    seleng.dma_start(out=st[:, 0:ni],
                                     in_=sel_d[:, coff:coff + ni])
                    call_tiles[k] = (m, st)

                for g in range(NGRP):
                    for t in range(TR):
                        nch = Mc[g][t]
                        c0 = boff[g][t] // 128
                        ps = None
                        first = True
                        if layer > 1 and g == 0:
                            ps = psum_seg.tile([128, W_], f32, tag="seg")
                            nc.tensor.matmul(ps[:], ones[:1, :128],
                                             Ws[b_name][:1, :], start=True,
                                             stop=(nch == 0))
                            first = False
                        if nch and ps is None:
                            ps = psum_seg.tile([128, W_], f32, tag="seg")
                        for j in range(nch):
                            cj = c0 + j
                            k = int(call_of_chunk[cj])
                            if k not in call_tiles:
                                emit_call(k)
                            m, st = call_tiles[k]
                            cc = cj - int(call_chunk0[k])
                            last = (j == nch - 1)
                            rhs = m[:, cc, :] if layer > 1 else m[:, cc, 0:D]
                            nc.tensor.matmul(ps[:],
                                             st[:, cc * 128:(cc + 1) * 128],
                                             rhs, start=first, stop=last)
                            first = False
                        # drain partial into SBUF accumulator
                        a = acc[:, t, :]
                        if ps is not None:
                            if g == 0:
                                nc.scalar.activation(a, ps[:], AF.Copy)
                            else:
                                nc.vector.tensor_add(out=a, in0=a, in1=ps[:])
                        elif g == 0:
                            nc.vector.memset(a, 0.0)
                        if g == NGRP - 1:
                            finish_tile(t, a)

            def to_xT(t, xt, xT_buf):
                tr = psum_tr.tile([128, H], bf16, tag="trb")
                nc.tensor.transpose(tr[:], xt[:], identb[:])
                nc.scalar.activation(xT_buf[:, t * 128:(t + 1) * 128], tr[:],
                                     AF.Copy)

            def make_stages(qn):
                stages = [stagep.tile([128, TQ, H], bf16, tag=f"st{q}",
                                      name=f"stage{qn}{q}")
                          for q in range(NGRP)]
                for t in range(TR, T):
                    nc.vector.memset(stages[t // TQ][:, t % TQ, :], 0.0)
                return stages

            def produce_next(t, xT_buf, Wn, stages, layer_next):
                hp = psum_h.tile([128, H], f32, tag="h")
                nc.tensor.matmul(hp[:], xT_buf[:, t * 128:(t + 1) * 128],
                                 Ws[Wn][:, :], start=True, stop=True)
                nc.scalar.activation(stages[t // TQ][:, t % TQ, :], hp[:],
                                     AF.Copy)
                import concourse.mybir as mybir
                for q in qfire.get(t, []):
                    nc.sync.dma_start(
                        out=hb[layer_next][q].ap().rearrange(
                            "(t p) f -> p t f", p=128),
                        in_=stages[q][:])
                    nc.gpsimd.collective_compute(
                        "AllGather", mybir.AluOpType.bypass,
                        replica_groups=[list(range(CORES))],
                        ins=[hb[layer_next][q].ap().opt()],
                        outs=[ha[layer_next][q].ap().opt()])

            for _rep in range(reps):
                # ================= layer 1 =================
                posq = [pos_pad_d[q * tq:(q + 1) * tq, :] for q in range(NGRP)]
                xT = xtp.tile([128, npc_pad], bf16, tag="xT")
                if T > TR:
                    nc.vector.memset(xT[:, TR * 128:], 0.0)
                stages2 = make_stages(2)

                def finish_l1(t, a, xT_buf=xT, stages=stages2):
                    aggT_ps = psum_tr.tile([128, 128], f32, tag="tr")
                    nc.tensor.transpose(aggT_ps[0:D, :], a, ident[:])
                    aggT = smallp.tile([D, 128], bf16, tag="aggTs")
                    nc.scalar.activation(aggT[:], aggT_ps[0:D, :], AF.Copy)
                    ps2 = psum_h.tile([128, H], f32, tag="h")
                    nc.tensor.matmul(ps2[:], aggT[:, :], Ws["W1"][:, :],
                                     start=True, stop=False)
                    nc.tensor.matmul(ps2[:], ones[:1, :128], Ws["b1"][:1, :],
                                     start=False, stop=True)
                    xt = xtilep.tile([128, H], bf16, tag="xt")
                    nc.scalar.activation(xt[:], ps2[:], AF.Relu)
                    to_xT(t, xt, xT_buf)
                    produce_next(t, xT_buf, "W2", stages, 2)

                edge_phase(1, posq, None, finish_l1)

                # ================= layer 2 =================
                xT2 = xtp.tile([128, npc_pad], bf16, tag="xT")
                if T > TR:
                    nc.vector.memset(xT2[:, TR * 128:], 0.0)
                stages3 = make_stages(3)

                def finish2(t, a, xT_buf=xT2, stages=stages3):
                    xt = xtilep.tile([128, H], bf16, tag="xt")
                    nc.scalar.activation(xt[:], a, AF.Relu)
                    to_xT(t, xt, xT_buf)
                    produce_next(t, xT_buf, "W3", stages, 3)

                edge_phase(2, ha[2], "b2", finish2)

                # ================= layer 3 =================
                pp = psum_h.tile([GP, H], f32, tag="h")

                def finish3(t, a, pp=pp):
                    xt = xtilep.tile([128, H], bf16, tag="xt")
                    nc.scalar.activation(xt[:], a, AF.Relu)
                    nc.tensor.matmul(pp[:], poolsel[:, t, :], xt[:],
                                     start=(t == 0), stop=(t == TR - 1))
                    if t == TR - 1:
                        psb = smallp.tile([GP, H], f32, tag="psb")
                        nc.scalar.activation(psb[:], pp[:], AF.Copy)
                        nc.sync.dma_start(out=pool_b[:, :], in_=psb[:])

                edge_phase(3, ha[3], "b3", finish3)

                # ================= pool + head =================
                import concourse.mybir as mybir
                nc.gpsimd.collective_compute(
                    "AllReduce", mybir.AluOpType.add,
                    replica_groups=[list(range(CORES))],
                    ins=[pool_b.ap().opt()], outs=[pool_r.ap().opt()])
                pooled = smallp.tile([GP, H], f32, tag="pooled")
                nc.sync.dma_start(out=pooled[:], in_=pool_r[:, :])
                gmean = smallp.tile([GP, H], bf16, tag="gmean")
                nc.scalar.activation(gmean[:], pooled[:], AF.Copy,
                                     scale=invcnt[:, 0:1])
                gT_ps = psum_tr.tile([128, GP], bf16, tag="trb")
                nc.tensor.transpose(gT_ps[:], gmean[:], identb[0:GP, 0:GP])
                gT = smallp.tile([H, GP], bf16, tag="gTs")
                nc.scalar.activation(gT[:], gT_ps[:, 0:GP], AF.Copy)
                hh_ps = psum_h.tile([GP, HH], f32, tag="h")
                nc.tensor.matmul(hh_ps[:], gT[:, :], Ws["Wl1"][:, :],
                                 start=True, stop=False)
                nc.tensor.matmul(hh_ps[:], ones[:1, 0:GP], Ws["bl1"][:1, :],
                                 start=False, stop=True)
                hh = smallp.tile([GP, HH], bf16, tag="hhs")
                nc.scalar.activation(hh[:], hh_ps[:], AF.Relu)
                hhT_ps = psum_tr.tile([HH, GP], bf16, tag="trb")
                nc.tensor.transpose(hhT_ps[:], hh[:], identb[0:GP, 0:GP])
                hhT = smallp.tile([HH, GP], bf16, tag="hhTs")
                nc.scalar.activation(hhT[:], hhT_ps[:], AF.Copy)
                o_ps = psum_h.tile([GP, HC], f32, tag="h")
                nc.tensor.matmul(o_ps[:], hhT[:, :], Ws["Wl2"][:, :],
                                 start=True, stop=False)
                nc.tensor.matmul(o_ps[:], ones[:1, 0:GP], Ws["bl2"][:1, :],
                                 start=False, stop=True)
                osb = smallp.tile([GP, HC], f32, tag="osb")
                nc.scalar.activation(osb[:], o_ps[:], AF.Copy)
                nc.sync.dma_start(out=out_d[:, :], in_=osb[:])

    nc.compile()
    return nc


# ----------------------------------------------------------------- entry

def kernel(pos, edge_index, batch, W1, b1, W2, b2, W3, b3, Wl1, bl1, Wl2, bl2,
           num_graphs):
    from concourse.bass_utils import run_bass_kernel_spmd

    pos = np.asarray(pos, dtype=np.float32)
    edge_index = np.asarray(edge_index)
    batch = np.asarray(batch)
    G = int(num_graphs)
    H = np.asarray(W2).shape[0]
    C = np.asarray(Wl2).shape[1]

    import sys, time as _time
    _t0 = _time.time()
    meta, data = _preprocess(pos, edge_index, batch, G)
    print(f"[kernel] preprocess done {_time.time()-_t0:.1f}s "
          f"tot={meta['tot']} ncalls={meta['ncalls']}",
          file=sys.stderr, flush=True)
    nc = _build(meta, H, C)
    print(f"[kernel] build+compile done {_time.time()-_t0:.1f}s",
          file=sys.stderr, flush=True)

    base = {
        "pos_pad": data["pos_pad"],
        "invcnt": data["invcnt"],
        "ident": np.eye(128, dtype=np.float32),
        "identb": np.eye(128, dtype=np.float32).astype(bf16_np),
        "ones": np.ones((1, 128), bf16_np),
        "W1": np.asarray(W1, np.float32).astype(bf16_np),
        "W2": np.asarray(W2, np.float32).astype(bf16_np),
        "W3": np.asarray(W3, np.float32).astype(bf16_np),
        "Wl1": np.asarray(Wl1, np.float32).astype(bf16_np),
        "Wl2": np.asarray(Wl2, np.float32).astype(bf16_np),
        "b1": np.asarray(b1, np.float32).reshape(1, -1).astype(bf16_np),
        "b2": np.asarray(b2, np.float32).reshape(1, -1).astype(bf16_np),
        "b3": np.asarray(b3, np.float32).reshape(1, -1).astype(bf16_np),
        "bl1": np.asarray(bl1, np.float32).reshape(1, -1).astype(bf16_np),
        "bl2": np.asarray(bl2, np.float32).reshape(1, -1).astype(bf16_np),
    }
    in_maps = []
    for c in range(CORES):
        m = dict(base)
        m["idxs"] = data["idxs"][c]
        m["sel"] = data["sel"][c]
        m["counts"] = data["counts"][c]
        m["poolsel"] = data["poolsel"][c]
        in_maps.append(m)

    print("[kernel] executing", file=sys.stderr, flush=True)
    res = run_bass_kernel_spmd(nc, in_maps, core_ids=list(range(CORES)))
    print(f"[kernel] exec done {_time.time()-_t0:.1f}s", file=sys.stderr,
          flush=True)
    global LAST_EXEC_NS, LAST_RESULT
    LAST_EXEC_NS = res.exec_time_ns
    LAST_RESULT = res
    out = res.results[0]["out"][:G].astype(np.float32)
    return out


LAST_EXEC_NS = None
LAST_RESULT = None
